# revision 1
# baseline (speedup 1.0000x reference)
# Trainium2 Bass kernel for the factorized-PC mixture likelihood:
#   phi = relu(z @ W1 + b1) @ W2 + b2                  (K, D)
#   sq[k,b] = ||phi_k||^2 + ||x_b||^2 - 2 phi_k . x_b  (K, B)
#   out = mean_b( sum_k w_k * exp(-sq[k,b]) )          scalar
#
# Sharding: data-parallel over the batch B across 8 cores (B=8192 -> 1024
# rows of x per core). Every core computes the full phi (redundant but tiny)
# and a partial sum over its batch slice; the 8 partial sums are combined on
# the host (sum / B). No collectives needed.
#
# Per core (b on partitions, k on the free axis):
#   G[b, k] = phi_k . x_b + 0.5*(ln w_k - ||phi_k||^2)   via PE matmuls:
#       - xT (D on partitions) via bf16 XBAR DMA transpose (DRAM round trip,
#         casts on the otherwise-idle gpsimd engine, triggers on idle SP)
#       - phiT (D on partitions) computed directly in transposed form
#       - ||phi_k||^2 as a quadratic form h~^T (W2aug W2aug^T) h~ so it
#         depends only on hT, not on phiT (the bias row lands early)
#       - one augmentation row (contraction length 1) adds the per-k terms
#   result = exp(2*G - ||x_b||^2) via one ACT pass per PSUM tile with
#       bias = -||x_b||^2 (per-partition), scale = 2.0, and accum_out
#       summing over the free (k) axis => mixture[b] per partition.
#   final scalar via ones-matmul partition reduction.
#
# The distance GEMM runs in bf16 (fp32 accumulate in PSUM). The exponent is
# O(-500) for these inputs, so exp underflows to 0 exactly as in the fp32
# reference; bf16 rounding of the exponent is far below the underflow margin.
#
# Built on Bacc (not plain Bass): its compile() pass splits multi-semaphore
# waits into EventSemaphore instructions - TRN2 allows 1 wait per instruction.

import numpy as np

import concourse.bass as bass
import concourse.bacc as bacc_mod
import concourse.mybir as mybir
from concourse.bass_utils import run_bass_kernel_spmd
from concourse.masks import make_identity
from concourse.tile import TileContext

N_CORES = 8
B, D, K, L, H = 8192, 512, 2048, 128, 64
BS = B // N_CORES  # 1024 batch rows per core

F32 = mybir.dt.float32
BF16 = mybir.dt.bfloat16
AF = mybir.ActivationFunctionType

KT = K // 128  # 16 k-tiles
BT = BS // 128  # 8 b-tiles per core
DT = D // 128  # 4 d-tiles
KC = K // 512  # 4 k-chunks of 512


def build_nc(ablate=()) -> bass.Bass:
    ablate = frozenset(ablate)
    nc = bacc_mod.Bacc("TRN2", target_bir_lowering=False)

    x_d = nc.dram_tensor("x", [BS, D], F32, kind="ExternalInput")
    z_d = nc.dram_tensor("z_samples", [K, L], F32, kind="ExternalInput")
    w_d = nc.dram_tensor("w", [K], F32, kind="ExternalInput")
    W1_d = nc.dram_tensor("W1", [L, H], F32, kind="ExternalInput")
    b1_d = nc.dram_tensor("b1", [H], F32, kind="ExternalInput")
    W2_d = nc.dram_tensor("W2", [H, D], F32, kind="ExternalInput")
    b2_d = nc.dram_tensor("b2", [D], F32, kind="ExternalInput")
    out_d = nc.dram_tensor("out", [1, 1], F32, kind="ExternalOutput")

    with TileContext(nc) as tc:
        with (
            tc.tile_pool(name="const", bufs=1) as cpool,
            tc.tile_pool(name="work", bufs=3) as wpool,
            tc.tile_pool(name="dram", bufs=1, space="DRAM") as dpool,
            tc.tile_pool(name="psA", bufs=4, space="PSUM") as psA,
            tc.tile_pool(name="psG", bufs=2, space="PSUM") as psG,
        ):
            # ---------------- constants ----------------
            # preload the one ACT table set covering Ln/Relu/Square/Exp/Copy
            # so the auto-inserter never needs a mid-kernel reload
            from concourse.hw_specs import get_activation_tables
            _set_id = list(get_activation_tables(nc.m.arch)).index(
                "natural_log_exp_and_others"
            )
            nc.scalar.add_instruction(
                mybir.InstLoadActFuncSet(
                    name=nc.get_next_instruction_name(),
                    ins=[],
                    outs=[],
                    act_func_set_id=_set_id,
                )
            )
            ident = cpool.tile([128, 128], F32)
            make_identity(nc, ident)
            ident_bf = cpool.tile([128, 128], BF16)
            nc.vector.tensor_copy(ident_bf, ident)
            ones_f32 = cpool.tile([128, 1], F32)
            nc.vector.memset(ones_f32, 1.0)
            neg1_bf = cpool.tile([128, 1], BF16)
            nc.vector.memset(neg1_bf, -1.0)
            half_bf = cpool.tile([1, 128], BF16)
            nc.vector.memset(half_bf, 0.5)

            # ---------------- input DMAs ----------------
            # tiny weight tensors first so weight prep isn't starved behind
            # the 3 MB of z/x traffic on the serial DMA path
            W2_sb = cpool.tile([H, D], F32)
            nc.sync.dma_start(W2_sb, W2_d[:, :])
            b2_row = cpool.tile([1, D], F32)
            nc.sync.dma_start(b2_row, b2_d[:].rearrange("(a d) -> a d", a=1))
            W1_sb = cpool.tile([L, H], F32)
            nc.sync.dma_start(W1_sb, W1_d[:, :])
            b1_col = cpool.tile([H, 1], F32)
            nc.sync.dma_start(b1_col, b1_d[:].rearrange("(h a) -> h a", a=1))
            w_row = cpool.tile([1, K], F32)
            nc.sync.dma_start(w_row, w_d[:].rearrange("(a k) -> a k", a=1))
            z_sb = cpool.tile([128, KT, L], F32)
            for zc in range(4):
                nc.sync.dma_start(
                    z_sb[:, 4 * zc : 4 * (zc + 1), :],
                    z_d[512 * zc : 512 * (zc + 1), :].rearrange("(t p) l -> p t l", p=128),
                )
            x_sb = cpool.tile([128, BT, D], F32)
            for t in range(BT):
                nc.sync.dma_start(x_sb[:, t, :], x_d[128 * t : 128 * (t + 1), :])

            # ---------------- ln w (first ACT op so the natural_log_exp
            # table set loads once, before relu/square traffic) ----------------
            lnw_row = cpool.tile([1, K], F32)
            nc.scalar.activation(lnw_row, w_row, AF.Ln)

            # ---------------- xT via bf16 DMA transpose ----------------
            # cast x to bf16 on gpsimd (keeps DVE/ACT free), round-trip
            # through DRAM with the XBAR transpose, pipelined in b-halves;
            # DMA triggers ride the otherwise idle SP queue.
            x_bf = cpool.tile([128, BT, D], BF16)
            x_bf_d = dpool.tile([BS, D], BF16)
            xT = cpool.tile([128, DT, BS], BF16)  # [dpart, dtile, b]
            NH = BT // 2
            for hb in range(2):
                for tt in range(NH):
                    t = NH * hb + tt
                    # second half: alternate gpsimd/DVE so the half-2 store
                    # (which gates the transposes) isn't paced by gpsimd alone
                    if hb == 1:
                        nc.vector.tensor_copy(x_bf[:, t, :], x_sb[:, t, :])
                    else:
                        nc.gpsimd.tensor_copy(x_bf[:, t, :], x_sb[:, t, :])
                rows = slice(512 * hb, 512 * (hb + 1))
                nc.sync.dma_start(
                    x_bf_d[rows, :].rearrange("(t p) d -> p t d", p=128),
                    x_bf[:, NH * hb : NH * (hb + 1), :],
                )
            for d in range(DT if "xT" not in ablate else 0):
                nc.sync.dma_start_transpose(xT[:, d, :], x_bf_d[:, 128 * d : 128 * (d + 1)])

            # ---------------- weight prep (bf16) ----------------
            W1_bf = cpool.tile([L, H], BF16)
            nc.vector.tensor_copy(W1_bf, W1_sb)
            # W2aug[:, d, :] = [W2[:, dslice]; b2[dslice]] -> lhsT with the
            # bias as a 65th contraction row (paired with the constant-1 row
            # appended to hT), so phi = W2.T h + b2 comes out of one matmul.
            W2aug = cpool.tile([H + 1, DT, 128], BF16)
            for d in range(DT):
                nc.vector.tensor_copy(W2aug[0:H, d, :], W2_sb[:, 128 * d : 128 * (d + 1)])
                nc.vector.tensor_copy(W2aug[H : H + 1, d, :], b2_row[:, 128 * d : 128 * (d + 1)])

            # W2aug transposed early (PE idle; feeds M for the p2 quadratic form)
            W2augT = cpool.tile([128, DT, H + 1], BF16)
            for d in range(DT):
                ptw = psA.tile([128, H + 1], BF16, tag="psA", name=f"ptw{d}")
                nc.tensor.transpose(ptw, W2aug[:, d, :], ident_bf[: H + 1, : H + 1])
                nc.vector.tensor_copy(W2augT[:, d, :], ptw)

            # ---------------- zT via PE transpose ----------------
            zT = cpool.tile([128, K], BF16)  # [l, k]
            for t in range(KT if "zT" not in ablate else 0):
                ptz = psA.tile([128, 128], F32, tag="psA", name=f"ptz{t}")
                nc.tensor.transpose(ptz, z_sb[:, t, :], ident)
                nc.vector.tensor_copy(zT[:, 128 * t : 128 * (t + 1)], ptz)

            # ---------------- M = W2aug @ W2aug^T (65x65) ----------------
            pm = psA.tile([H + 1, H + 1], F32, tag="psA", name="pm")
            for d in range(DT):
                nc.tensor.matmul(
                    pm, W2augT[:, d, :], W2augT[:, d, :], start=(d == 0), stop=(d == DT - 1)
                )
            M_bf = cpool.tile([H + 1, H + 1], BF16)
            nc.vector.tensor_copy(M_bf, pm)

            # ---------------- hT = relu(W1.T zT + b1), plus ones row ----------------
            hTaug = cpool.tile([H + 1, K], BF16)
            nc.gpsimd.memset(hTaug[H : H + 1, :], 1.0)
            for c in range(KC):
                ph = psA.tile([H, 512], F32, tag="psA", name=f"ph{c}")
                nc.tensor.matmul(ph, W1_bf, zT[:, 512 * c : 512 * (c + 1)], start=True, stop=True)
                nc.scalar.activation(
                    hTaug[0:H, 512 * c : 512 * (c + 1)], ph, AF.Relu, bias=b1_col, scale=1.0
                )

            # ---------------- Mh + phiT, interleaved per k-chunk ----------------
            # Mh = M @ hTaug feeds p2; phiT = W2aug^T hTaug feeds the main GEMM.
            # Both consume hTaug chunk-by-chunk right after each relu lands.
            Mh = cpool.tile([H + 1, K], BF16)
            phiT = cpool.tile([128, DT, K], BF16)
            for c in range(KC):
                pmh = psA.tile([H + 1, 512], F32, tag="psA", name=f"pmh{c}")
                nc.tensor.matmul(
                    pmh, M_bf, hTaug[:, 512 * c : 512 * (c + 1)], start=True, stop=True
                )
                nc.scalar.copy(Mh[:, 512 * c : 512 * (c + 1)], pmh)
                for d in range(DT if "phi" not in ablate else 0):
                    pp = psA.tile([128, 512], F32, tag="psA", name=f"pp{d}_{c}")
                    nc.tensor.matmul(
                        pp, W2aug[:, d, :], hTaug[:, 512 * c : 512 * (c + 1)], start=True, stop=True
                    )
                    dst = phiT[:, d, 512 * c : 512 * (c + 1)]
                    if d % 2 == 0:
                        nc.vector.tensor_copy(dst, pp)
                    else:
                        nc.scalar.copy(dst, pp)

            # ---------------- biasrow = ln w - p2 ----------------
            # qf = h~ * Mh elementwise; p2 = column-sum(qf) via (-1)-ones matmul
            qf = cpool.tile([H + 1, K], BF16)
            nc.vector.tensor_mul(qf, hTaug, Mh)
            biasrow = cpool.tile([1, K], BF16)
            for c in range(KC):
                pq = psA.tile([1, 512], F32, tag="psA", name=f"pq{c}")
                nc.tensor.matmul(
                    pq, neg1_bf[: H + 1], qf[:, 512 * c : 512 * (c + 1)], start=True, stop=True
                )
                nc.vector.tensor_tensor(
                    biasrow[:, 512 * c : 512 * (c + 1)],
                    lnw_row[:, 512 * c : 512 * (c + 1)],
                    pq,
                    mybir.AluOpType.add,
                )

            x2pos = cpool.tile([128, BT], F32)
            negx2 = cpool.tile([128, BT], F32)
            if "x2" in ablate:
                nc.vector.memset(negx2, 0.0)

            # ---------------- main GEMM + fused exp/reduce ----------------
            # G[b,k] accumulated over 4 d-tiles plus the augmentation row;
            # ACT computes exp(2*G - x2) and accumulates over k per partition.
            Racc = cpool.tile([128, 2 * BT], F32)
            if "main" in ablate:
                nc.vector.memset(Racc, 0.0)
            else:
                for t in range(BT):
                    if "x2" not in ablate:
                        # per-tile ||x_b||^2: fills the ACT idle slot in the
                        # PE-paced exp cadence
                        xsq = wpool.tile([128, D], BF16, tag="xsq", name=f"xsq{t}")
                        nc.scalar.activation(
                            xsq, x_sb[:, t, :], AF.Square, accum_out=x2pos[:, t : t + 1]
                        )
                        nc.gpsimd.tensor_scalar_mul(
                            negx2[:, t : t + 1], x2pos[:, t : t + 1], -1.0
                        )
                    for hlf in range(2):  # halves of K: 1024 columns each
                        pg = psG.tile([128, 1024], F32, tag="psG", name=f"pg{t}_{hlf}")
                        for d in range(DT):
                            for c2 in range(2):
                                kofs = 1024 * hlf + 512 * c2
                                nc.tensor.matmul(
                                    pg[:, 512 * c2 : 512 * (c2 + 1)],
                                    xT[:, d, 128 * t : 128 * (t + 1)],
                                    phiT[:, d, kofs : kofs + 512],
                                    start=(d == 0),
                                    stop=False,
                                )
                        for c2 in range(2):
                            kofs = 1024 * hlf + 512 * c2
                            nc.tensor.matmul(
                                pg[:, 512 * c2 : 512 * (c2 + 1)],
                                half_bf,
                                biasrow[:, kofs : kofs + 512],
                                start=False,
                                stop=True,
                            )
                        if "exp" in ablate:
                            nc.vector.memset(Racc[:, 2 * t + hlf : 2 * t + hlf + 1], 0.0)
                        else:
                            U = wpool.tile([128, 1024], BF16, tag="U", name=f"U{t}_{hlf}")
                            nc.scalar.activation(
                                U,
                                pg,
                                AF.Exp,
                                bias=negx2[:, t : t + 1],
                                scale=2.0,
                                accum_out=Racc[:, 2 * t + hlf : 2 * t + hlf + 1],
                            )

            # ---------------- final reduction to one scalar ----------------
            sps = psA.tile([1, 2 * BT], F32, tag="psA")
            nc.tensor.matmul(sps, ones_f32, Racc, start=True, stop=True)
            total_sb = cpool.tile([1, 1], F32)
            nc.vector.tensor_reduce(
                total_sb, sps, axis=mybir.AxisListType.X, op=mybir.AluOpType.add
            )
            nc.sync.dma_start(out_d[:, :], total_sb)

    nc.finalize()
    return nc


_NC_CACHE = None


def _get_nc() -> bass.Bass:
    global _NC_CACHE
    if _NC_CACHE is None:
        _NC_CACHE = build_nc()
    return _NC_CACHE


def kernel(x, z_samples, w, W1, b1, W2, b2, _trace=False):
    x = np.ascontiguousarray(np.asarray(x, dtype=np.float32))
    z_samples = np.ascontiguousarray(np.asarray(z_samples, dtype=np.float32))
    w = np.ascontiguousarray(np.asarray(w, dtype=np.float32))
    W1 = np.ascontiguousarray(np.asarray(W1, dtype=np.float32))
    b1 = np.ascontiguousarray(np.asarray(b1, dtype=np.float32))
    W2 = np.ascontiguousarray(np.asarray(W2, dtype=np.float32))
    b2 = np.ascontiguousarray(np.asarray(b2, dtype=np.float32))

    nc = _get_nc()
    in_maps = [
        {
            "x": x[i * BS : (i + 1) * BS],
            "z_samples": z_samples,
            "w": w,
            "W1": W1,
            "b1": b1,
            "W2": W2,
            "b2": b2,
        }
        for i in range(N_CORES)
    ]
    res = run_bass_kernel_spmd(nc, in_maps, core_ids=list(range(N_CORES)), trace=_trace)
    total = sum(float(r["out"][0, 0]) for r in res.results)
    out = np.array(total / B, dtype=np.float32)
    if _trace:
        return out, res
    return out



# revision 37
# speedup vs baseline: 1.5029x; 1.5029x over previous
# Trainium2 Bass kernel for the factorized-PC mixture likelihood:
#   phi = relu(z @ W1 + b1) @ W2 + b2                  (K, D)
#   sq[k,b] = ||phi_k||^2 + ||x_b||^2 - 2 phi_k . x_b  (K, B)
#   out = mean_b( sum_k w_k * exp(-sq[k,b]) )          scalar
#
# Sharding: data-parallel over batch B across 8 cores (1024 rows of x per
# core); every core computes the full h (tiny) and partial per-row mixture
# sums; host combines (sum / B). No collectives.
#
# Key reassociation: G = x . phi^T = (x . W2aug^T) . h~ -- contracting over
# the 65-dim hidden space instead of the 512-dim feature space. With
#   y~T = [W2aug . x^T; 1]   (66, B)   [W2aug = [W2; b2]]
#   h~  = [relu(zW1+b1); 1; 0.5(ln w - p2)]  (66, K)
# the main GEMM per (b-tile, k-chunk) is ONE 66-row bf16 matmul whose PSUM
# directly holds G = phi.x + 0.5(ln w - p2). ACT computes exp(2G - x2_b)
# with accum_out giving per-row mixture sums.
#
#  * p2 via the quadratic form h~^T (W2aug W2aug^T) h~: qf consumes the Mh
#    PSUM directly; the column-sum matmul uses -0.5/+0.5 weights and folds
#    in ln w, so the bias row comes from one PE pass + one ACT copy.
#  * x path: DMA f32 -> bf16 cast -> PE transposes -> xT tile; then
#    y~T[:, btile] = sum_d W2augT[d]^T xT[d] lands transposed in one PSUM
#    accumulation (no extra transpose round trip).
#  * x2 = -||x||^2 fused per tile: (x * -1) * x with accum_out (one op).
#  * PSUM: four 1-bank "sm" slots (prep) + two 2-bank "big" slots (G).
#  * emission order == intended execution order (the per-engine streams
#    largely follow it), with late x tiles interleaved between main passes.
#  * per-row results [128, 16] are DMA'd out; the host sums and divides
#    by B.
#
# The exponent is O(-500) for these inputs, so exp underflows to 0 exactly
# as in the fp32 reference; bf16 rounding of the exponent is far below the
# underflow margin.
#
# Built on Bacc: its compile() pass splits multi-semaphore waits into
# EventSemaphore instructions (TRN2 allows 1 wait per instruction).

import numpy as np

import concourse.bass as bass
import concourse.bacc as bacc_mod
import concourse.mybir as mybir
from concourse.bass_utils import run_bass_kernel_spmd
from concourse.masks import make_identity
from concourse.tile import TileContext

N_CORES = 8
B, D, K, L, H = 8192, 512, 2048, 128, 64
BS = B // N_CORES  # 1024 batch rows per core

F32 = mybir.dt.float32
BF16 = mybir.dt.bfloat16
AF = mybir.ActivationFunctionType

KT = K // 128  # 16 k-tiles
BT = BS // 128  # 8 b-tiles per core
DT = D // 128  # 4 d-tiles
KC = K // 512  # 4 k-chunks of 512
HA = H + 1  # 65: hidden dims + ones row (pairs W2aug's b2 row)
HE = H + 2  # 66: + bias row (0.5(ln w - p2)) / ones row on the y side


def build_nc() -> bass.Bass:
    nc = bacc_mod.Bacc("TRN2", target_bir_lowering=False)

    x_d = nc.dram_tensor("x", [BS, D], F32, kind="ExternalInput")
    z_d = nc.dram_tensor("z_samples", [K, L], F32, kind="ExternalInput")
    w_d = nc.dram_tensor("w", [K], F32, kind="ExternalInput")
    W1_d = nc.dram_tensor("W1", [L, H], F32, kind="ExternalInput")
    b1_d = nc.dram_tensor("b1", [H], F32, kind="ExternalInput")
    W2_d = nc.dram_tensor("W2", [H, D], F32, kind="ExternalInput")
    b2_d = nc.dram_tensor("b2", [D], F32, kind="ExternalInput")
    out_d = nc.dram_tensor("out", [128, 2 * BT], F32, kind="ExternalOutput")

    with TileContext(nc) as tc:
        with (
            tc.tile_pool(name="const", bufs=1) as cpool,
            tc.tile_pool(name="work", bufs=2) as wpool,
            tc.tile_pool(name="ps", bufs=1, space="PSUM") as psp,
        ):
            # ---------------- constants ----------------
            from concourse.hw_specs import get_activation_tables
            _set_id = list(get_activation_tables(nc.m.arch)).index(
                "natural_log_exp_and_others"
            )
            nc.scalar.add_instruction(
                mybir.InstLoadActFuncSet(
                    name=nc.get_next_instruction_name(),
                    ins=[],
                    outs=[],
                    act_func_set_id=_set_id,
                )
            )
            ident = cpool.tile([128, 128], F32)
            make_identity(nc, ident)
            # PE p-state warmup: ~3us of cumulative PE busy unlocks the full
            # clock; burn it on junk transposes while the first DMAs land
            warm = psp.tile([128, 128], F32, tag="sm", bufs=4, name="warm")
            for _ in range(9):
                nc.tensor.transpose(warm, ident, ident)
            ident_bf = cpool.tile([128, 128], BF16)
            nc.vector.tensor_copy(ident_bf, ident)
            # bias-row builders (engine APs may only start at partitions
            # 0/32/64/96, so the bias lands as a 2-row [ones; bias] block at
            # partition 64): pb2 row0 = 1, row1 = 0.5(ln w - p2)
            ones_row = cpool.tile([1, 512], BF16)
            nc.gpsimd.memset(ones_row, 1.0)
            onesel = cpool.tile([1, 2], BF16)
            nc.gpsimd.memset(onesel[:, 0:1], 1.0)
            nc.gpsimd.memset(onesel[:, 1:2], 0.0)
            possel = cpool.tile([1, 2], BF16)
            nc.gpsimd.memset(possel[:, 0:1], 0.0)
            nc.gpsimd.memset(possel[:, 1:2], 0.5)
            neghalf2 = cpool.tile([HA, 2], BF16)
            nc.gpsimd.memset(neghalf2, 0.0)
            nc.gpsimd.memset(neghalf2[:, 1:2], -0.5)

            # ---------------- input DMAs ----------------
            # HWDGE queue (625ns/op serial): z half 0 first (longest chain),
            # W1/b1, x quarter 0, z half 1, x q1..q3. Small weights (W2, w,
            # b2) go through the Pool SWDGE path in parallel.
            z_sb = cpool.tile([128, KT, L], F32)
            nc.sync.dma_start(
                z_sb[:, 0:8, :],
                z_d[0:1024, :].rearrange("(t p) l -> p t l", p=128),
            )
            w_row = cpool.tile([1, K], F32)
            nc.sync.dma_start(w_row, w_d[:].rearrange("(a k) -> a k", a=1))
            W1_sb = cpool.tile([L, H], F32)
            nc.sync.dma_start(W1_sb, W1_d[:, :])
            b2_row = cpool.tile([1, D], F32)
            nc.sync.dma_start(b2_row, b2_d[:].rearrange("(a d) -> a d", a=1))
            b1_col = cpool.tile([H, 1], F32)
            nc.sync.dma_start(b1_col, b1_d[:].rearrange("(h a) -> h a", a=1))
            x_sb = cpool.tile([128, BT, D], F32)
            nc.sync.dma_start(
                x_sb[:, 0:2, :],
                x_d[0:256, :].rearrange("(t p) d -> p t d", p=128),
            )
            nc.sync.dma_start(
                z_sb[:, 8:16, :],
                z_d[1024:2048, :].rearrange("(t p) l -> p t l", p=128),
            )
            for q in range(1, 4):
                nc.sync.dma_start(
                    x_sb[:, 2 * q : 2 * (q + 1), :],
                    x_d[256 * q : 256 * (q + 1), :].rearrange("(t p) d -> p t d", p=128),
                )
            # SWDGE (Pool) path just for W2 (feeds the M chain in parallel)
            W2_sb = cpool.tile([H, D], F32)
            nc.gpsimd.dma_start(W2_sb, W2_d[:, :])

            # ---------------- persistent tensors ----------------
            zT = cpool.tile([128, KT, L], BF16)  # [l, kt, k]
            W2aug = cpool.tile([HA, DT, 128], BF16)
            W2augT = cpool.tile([128, DT, HA], BF16)
            M_bf = cpool.tile([HA, HA], BF16)
            W1_bf = cpool.tile([L, H], BF16)
            qfaug = cpool.tile([HA, K], BF16)  # rows 0..64 = h~ .* Mh~
            lnw_row = cpool.tile([1, K], BF16)  # ln w
            hTaug = cpool.tile([HE, K], BF16)  # relu; ones; 0.5(ln w - p2)
            yTaug = cpool.tile([HE, BS], BF16)  # [j, b]; row 65 ones
            negx2 = cpool.tile([128, BT], F32)
            racc = cpool.tile([128, 2 * BT], F32)

            # ---------------- emission in expected execution order ---------
            # Per-engine instruction streams run strictly in emission order,
            # so ops are emitted in the order they are expected to become
            # ready: z chunk chains as z halves land, W2aug/M prep between,
            # x tiles as x quarters land, then main-loop passes interleaved
            # with the remaining x tiles.
            relu_eng = {1: nc.vector, 2: nc.vector, 3: nc.vector}
            zt_eng = {0: nc.scalar, 1: nc.vector, 2: nc.vector, 3: nc.vector}
            qf_eng = {0: nc.vector, 1: nc.vector, 2: nc.vector, 3: nc.vector}

            def z_transpose(c):
                ptz = psp.tile([128, 4, 128], F32, tag="sm", bufs=4, name=f"ptz{c}")
                for j in range(4):
                    nc.tensor.transpose(ptz[:, j, :], z_sb[:, 4 * c + j, :], ident)
                if c == 0:
                    nc.scalar.copy(zT[:, 4 * c : 4 * (c + 1), :], ptz)
                else:
                    zt_eng[c].tensor_copy(zT[:, 4 * c : 4 * (c + 1), :], ptz)

            def h_chunk(c):
                ks = slice(512 * c, 512 * (c + 1))
                ph = psp.tile([H, 512], F32, tag="sm", bufs=4, name=f"ph{c}")
                nc.tensor.matmul(
                    ph, W1_bf, zT[:, 4 * c : 4 * (c + 1), :], start=True, stop=True
                )
                if c == 0:
                    nc.scalar.activation(
                        hTaug[0:H, ks], ph, AF.Relu, bias=b1_col, scale=1.0
                    )
                else:
                    relu_eng[c].tensor_scalar(
                        hTaug[0:H, ks], ph, b1_col, 0.0,
                        op0=mybir.AluOpType.add, op1=mybir.AluOpType.max,
                    )

            def bias_chunk(c):
                ks = slice(512 * c, 512 * (c + 1))
                pmh = psp.tile([HA, 512], F32, tag="sm", bufs=4, name=f"pmh{c}")
                nc.tensor.matmul(pmh, M_bf, hTaug[0:HA, ks], start=True, stop=True)
                qf_eng[c].scalar_tensor_tensor(
                    qfaug[:, ks], pmh, 1.0, hTaug[0:HA, ks],
                    op0=mybir.AluOpType.mult, op1=mybir.AluOpType.mult,
                )
                pb2 = psp.tile([2, 512], F32, tag="sm", bufs=4, name=f"pb2_{c}")
                nc.tensor.matmul(pb2, onesel, ones_row, start=True, stop=False)
                nc.tensor.matmul(pb2, neghalf2, qfaug[:, ks], start=False, stop=False)
                nc.tensor.matmul(pb2, possel, lnw_row[:, ks], start=False, stop=True)
                if c < 2:
                    nc.scalar.copy(hTaug[H:HE, ks], pb2)
                elif c == 2:
                    nc.vector.tensor_copy(hTaug[H:HE, ks], pb2)
                else:
                    nc.vector.tensor_copy(hTaug[H:HE, ks], pb2)

            def x_tile(t):
                xsq = wpool.tile([128, D], BF16, tag="xsq", name=f"xsq{t}")
                x16 = wpool.tile([128, D], BF16, tag="x16", name=f"x16_{t}")
                nc.vector.tensor_copy(x16, x_sb[:, t, :])
                ptx = psp.tile([128, DT, 128], BF16, tag="sm", bufs=4, name=f"ptx{t}")
                for d in range(DT):
                    nc.tensor.transpose(
                        ptx[:, d, :], x16[:, 128 * d : 128 * (d + 1)], ident_bf
                    )
                xTt = wpool.tile([128, DT, 128], BF16, tag="xT", name=f"xT{t}")
                nc.vector.tensor_copy(xTt, ptx)
                pyT = psp.tile([HA, 128], F32, tag="sm", bufs=4, name=f"pyT{t}")
                for d in range(DT):
                    nc.tensor.matmul(
                        pyT, W2augT[:, d, :], xTt[:, d, :],
                        start=(d == 0), stop=(d == DT - 1),
                    )
                nc.vector.tensor_copy(yTaug[0:HA, 128 * t : 128 * (t + 1)], pyT)
                # -x2 fused: (x * -1) * x with accumulate
                nc.vector.scalar_tensor_tensor(
                    xsq, x_sb[:, t, :], -1.0, x_sb[:, t, :],
                    op0=mybir.AluOpType.mult, op1=mybir.AluOpType.mult,
                    accum_out=negx2[:, t : t + 1],
                )

            def x_pair(t):
                # two b-tiles share one PSUM bank for transposes and one for
                # yT: halves the "sm" slot pressure for the late tiles
                x16 = wpool.tile([128, 2, D], BF16, tag="x16p", name=f"x16p{t}")
                nc.gpsimd.tensor_copy(x16[:, 0, :], x_sb[:, t, :])
                nc.gpsimd.tensor_copy(x16[:, 1, :], x_sb[:, t + 1, :])
                ptx = psp.tile([128, 2, DT, 128], BF16, tag="sm", bufs=4, name=f"ptxp{t}")
                for j in range(2):
                    for d in range(DT):
                        nc.tensor.transpose(
                            ptx[:, j, d, :], x16[:, j, 128 * d : 128 * (d + 1)], ident_bf
                        )
                xTt = wpool.tile([128, 2, DT, 128], BF16, tag="xTp", name=f"xTp{t}")
                nc.vector.tensor_copy(xTt, ptx)
                pyT = psp.tile([HA, 2, 128], F32, tag="sm", bufs=4, name=f"pyTp{t}")
                for j in range(2):
                    for d in range(DT):
                        nc.tensor.matmul(
                            pyT[:, j, :], W2augT[:, d, :], xTt[:, j, d, :],
                            start=(d == 0), stop=(d == DT - 1),
                        )
                nc.vector.tensor_copy(yTaug[0:HA, 128 * t : 128 * (t + 2)], pyT)
                for j in range(2):
                    xsq = wpool.tile([128, D], BF16, tag="xsq", name=f"xsq{t+j}")
                    nc.vector.scalar_tensor_tensor(
                        xsq, x_sb[:, t + j, :], -1.0, x_sb[:, t + j, :],
                        op0=mybir.AluOpType.mult, op1=mybir.AluOpType.mult,
                        accum_out=negx2[:, t + j : t + j + 1],
                    )

            def main_pass(hlf, t):
                bsl = slice(128 * t, 128 * (t + 1))
                pg = psp.tile([128, 1024], F32, tag="big", bufs=2, name=f"pg{t}_{hlf}")
                for c2 in range(2):
                    c = 2 * hlf + c2
                    ks = slice(512 * c, 512 * (c + 1))
                    nc.tensor.matmul(
                        pg[:, 512 * c2 : 512 * (c2 + 1)],
                        yTaug[:, bsl],
                        hTaug[:, ks],
                        start=True,
                        stop=True,
                    )
                U = wpool.tile([128, 1024], BF16, tag="U", name=f"U{t}_{hlf}")
                nc.scalar.activation(
                    U,
                    pg,
                    AF.Exp,
                    bias=negx2[:, t : t + 1],
                    scale=2.0,
                    accum_out=racc[:, 2 * t + hlf : 2 * t + hlf + 1],
                )

            # ones rows: no upstream deps. yTaug rows 64:66 start as ones;
            # the per-tile yT copies later overwrite row 64 with x.b2
            nc.vector.memset(hTaug[H : H + 1, :], 1.0)
            nc.gpsimd.memset(yTaug[H:HE, :], 1.0)

            # t~2.1: z half 0 -> chunk 0 transposes; W1 cast
            z_transpose(0)
            nc.vector.tensor_copy(W1_bf, W1_sb)
            # W-block: W2aug -> W2augT -> M (W2 ~2.5 SWDGE, b2 ~3.4 HWDGE)
            nc.vector.tensor_copy(
                W2aug[0:H, :, :], W2_sb.rearrange("h (t d) -> h t d", d=128)
            )
            nc.vector.tensor_copy(
                W2aug[H:HA, :, :], b2_row.rearrange("a (t d) -> a t d", d=128)
            )
            # 66-wide so each bf16 d-slice stays 4-byte aligned in PSUM
            ptw = psp.tile([128, DT, HA + 1], BF16, tag="sm", bufs=4, name="ptw")
            for d in range(DT):
                nc.tensor.transpose(ptw[:, d, 0:HA], W2aug[:, d, :], ident_bf[:HA, :HA])
            nc.vector.tensor_copy(W2augT, ptw[:, :, 0:HA])
            h_chunk(0)
            pm = psp.tile([HA, HA], F32, tag="sm", bufs=4, name="pm")
            for d in range(DT):
                nc.tensor.matmul(
                    pm, W2augT[:, d, :], W2augT[:, d, :], start=(d == 0), stop=(d == DT - 1)
                )
            nc.vector.tensor_copy(M_bf, pm)
            nc.scalar.activation(
                lnw_row[:, 0:1024], w_row[:, 0:1024], AF.Ln
            )
            z_transpose(1)
            h_chunk(1)
            bias_chunk(0)
            x_tile(0)
            bias_chunk(1)
            x_tile(1)
            nc.scalar.activation(
                lnw_row[:, 1024:2048], w_row[:, 1024:2048], AF.Ln
            )
            main_pass(0, 0)
            # t~8.3: z half 1 -> chunks 2/3 (bias copies off ACT: it exps now)
            x_pair(2)
            main_pass(0, 1)
            z_transpose(2)
            h_chunk(2)
            x_pair(4)
            main_pass(0, 2)
            z_transpose(3)
            h_chunk(3)
            x_pair(6)
            main_pass(0, 3)
            bias_chunk(2)
            main_pass(0, 4)
            bias_chunk(3)
            main_pass(0, 5)
            main_pass(0, 6)
            main_pass(0, 7)
            for t in range(BT):
                main_pass(1, t)

            # ---------------- ship per-row sums; host sums / B ----------------
            # split so most of the DMA overlaps the last exp passes
            nc.sync.dma_start(out_d[:, 0:8], racc[:, 0:8])
            nc.sync.dma_start(out_d[:, 8:16], racc[:, 8:16])

    nc.finalize()
    return nc


_NC_CACHE = None


def _get_nc() -> bass.Bass:
    global _NC_CACHE
    if _NC_CACHE is None:
        _NC_CACHE = build_nc()
    return _NC_CACHE


def kernel(x, z_samples, w, W1, b1, W2, b2, _trace=False):
    x = np.ascontiguousarray(np.asarray(x, dtype=np.float32))
    z_samples = np.ascontiguousarray(np.asarray(z_samples, dtype=np.float32))
    w = np.ascontiguousarray(np.asarray(w, dtype=np.float32))
    W1 = np.ascontiguousarray(np.asarray(W1, dtype=np.float32))
    b1 = np.ascontiguousarray(np.asarray(b1, dtype=np.float32))
    W2 = np.ascontiguousarray(np.asarray(W2, dtype=np.float32))
    b2 = np.ascontiguousarray(np.asarray(b2, dtype=np.float32))

    nc = _get_nc()
    in_maps = [
        {
            "x": x[i * BS : (i + 1) * BS],
            "z_samples": z_samples,
            "w": w,
            "W1": W1,
            "b1": b1,
            "W2": W2,
            "b2": b2,
        }
        for i in range(N_CORES)
    ]
    res = run_bass_kernel_spmd(nc, in_maps, core_ids=list(range(N_CORES)), trace=_trace)
    total = sum(float(np.sum(np.asarray(r["out"], dtype=np.float64))) for r in res.results)
    out = np.array(total / B, dtype=np.float32)
    if _trace:
        return out, res
    return out


# revision 43
# speedup vs baseline: 1.5316x; 1.0192x over previous
# Trainium2 Bass kernel for the factorized-PC mixture likelihood:
#   phi = relu(z @ W1 + b1) @ W2 + b2                  (K, D)
#   sq[k,b] = ||phi_k||^2 + ||x_b||^2 - 2 phi_k . x_b  (K, B)
#   out = mean_b( sum_k w_k * exp(-sq[k,b]) )          scalar
#
# Sharding: data-parallel over batch B across 8 cores (1024 rows of x per
# core); every core computes the full h (tiny) and partial per-row mixture
# sums; host combines (sum / B). No collectives.
#
# Key reassociation: G = x . phi^T = (x . W2aug^T) . h~ -- contracting over
# the 65-dim hidden space instead of the 512-dim feature space. With
#   y~T = [W2aug . x^T; 1]   (66, B)   [W2aug = [W2; b2]]
#   h~  = [relu(zW1+b1); 1; 0.5(ln w - p2)]  (66, K)
# the main GEMM per (b-tile, k-chunk) is ONE 66-row bf16 matmul whose PSUM
# directly holds G = phi.x + 0.5(ln w - p2). ACT computes exp(2G - x2_b)
# with accum_out giving per-row mixture sums.
#
#  * p2 via the quadratic form h~^T (W2aug W2aug^T) h~: qf consumes the Mh
#    PSUM directly; the column-sum matmul uses -0.5/+0.5 weights and folds
#    in ln w, so the bias row comes from one PE pass + one ACT copy.
#  * x path: DMA f32 -> bf16 cast -> PE transposes -> xT tile; then
#    y~T[:, btile] = sum_d W2augT[d]^T xT[d] lands transposed in one PSUM
#    accumulation (no extra transpose round trip).
#  * x2 = -||x||^2 fused per tile: (x * -1) * x with accum_out (one op).
#  * PSUM: four 1-bank "sm" slots (prep) + two 2-bank "big" slots (G).
#  * emission order == intended execution order (the per-engine streams
#    largely follow it), with late x tiles interleaved between main passes.
#  * per-row results [128, 16] are DMA'd out; the host sums and divides
#    by B.
#
# The exponent is O(-500) for these inputs, so exp underflows to 0 exactly
# as in the fp32 reference; bf16 rounding of the exponent is far below the
# underflow margin.
#
# Built on Bacc: its compile() pass splits multi-semaphore waits into
# EventSemaphore instructions (TRN2 allows 1 wait per instruction).

import numpy as np

import concourse.bass as bass
import concourse.bacc as bacc_mod
import concourse.mybir as mybir
from concourse.bass_utils import run_bass_kernel_spmd
from concourse.masks import make_identity
from concourse.tile import TileContext

N_CORES = 8
B, D, K, L, H = 8192, 512, 2048, 128, 64
BS = B // N_CORES  # 1024 batch rows per core

F32 = mybir.dt.float32
BF16 = mybir.dt.bfloat16
AF = mybir.ActivationFunctionType

KT = K // 128  # 16 k-tiles
BT = BS // 128  # 8 b-tiles per core
DT = D // 128  # 4 d-tiles
KC = K // 512  # 4 k-chunks of 512
HA = H + 1  # 65: hidden dims + ones row (pairs W2aug's b2 row)
HE = H + 2  # 66: + bias row (0.5(ln w - p2)) / ones row on the y side


def build_nc() -> bass.Bass:
    nc = bacc_mod.Bacc("TRN2", target_bir_lowering=False)

    x_d = nc.dram_tensor("x", [BS, D], F32, kind="ExternalInput")
    z_d = nc.dram_tensor("z_samples", [K, L], F32, kind="ExternalInput")
    w_d = nc.dram_tensor("w", [K], F32, kind="ExternalInput")
    W1_d = nc.dram_tensor("W1", [L, H], F32, kind="ExternalInput")
    b1_d = nc.dram_tensor("b1", [H], F32, kind="ExternalInput")
    W2_d = nc.dram_tensor("W2", [H, D], F32, kind="ExternalInput")
    b2_d = nc.dram_tensor("b2", [D], F32, kind="ExternalInput")
    out_d = nc.dram_tensor("out", [128, 2 * BT], F32, kind="ExternalOutput")

    with TileContext(nc) as tc:
        with (
            tc.tile_pool(name="const", bufs=1) as cpool,
            tc.tile_pool(name="work", bufs=2) as wpool,
            tc.tile_pool(name="ps", bufs=1, space="PSUM") as psp,
        ):
            # ---------------- constants ----------------
            from concourse.hw_specs import get_activation_tables
            _set_id = list(get_activation_tables(nc.m.arch)).index(
                "natural_log_exp_and_others"
            )
            nc.scalar.add_instruction(
                mybir.InstLoadActFuncSet(
                    name=nc.get_next_instruction_name(),
                    ins=[],
                    outs=[],
                    act_func_set_id=_set_id,
                )
            )
            ident = cpool.tile([128, 128], F32)
            make_identity(nc, ident)
            # PE p-state warmup: ~3us of cumulative PE busy unlocks the full
            # clock; burn it on junk transposes while the first DMAs land
            warm = psp.tile([128, 128], F32, tag="sm", bufs=4, name="warm")
            for _ in range(9):
                nc.tensor.transpose(warm, ident, ident)
            ident_bf = cpool.tile([128, 128], BF16)
            nc.vector.tensor_copy(ident_bf, ident)
            # bias-row builders (engine APs may only start at partitions
            # 0/32/64/96, so the bias lands as a 2-row [ones; bias] block at
            # partition 64): pb2 row0 = 1, row1 = 0.5(ln w - p2)
            ones_row = cpool.tile([1, 512], BF16)
            nc.gpsimd.memset(ones_row, 1.0)
            onesel = cpool.tile([1, 2], BF16)
            nc.gpsimd.memset(onesel[:, 0:1], 1.0)
            nc.gpsimd.memset(onesel[:, 1:2], 0.0)
            possel = cpool.tile([1, 2], BF16)
            nc.gpsimd.memset(possel[:, 0:1], 0.0)
            nc.gpsimd.memset(possel[:, 1:2], 0.5)
            neghalf2 = cpool.tile([HA, 2], BF16)
            nc.gpsimd.memset(neghalf2, 0.0)
            nc.gpsimd.memset(neghalf2[:, 1:2], -0.5)

            # ---------------- input DMAs ----------------
            # HWDGE queue (625ns/op serial): z half 0 first (longest chain),
            # W1/b1, x quarter 0, z half 1, x q1..q3. Small weights (W2, w,
            # b2) go through the Pool SWDGE path in parallel.
            z_sb = cpool.tile([128, KT, L], F32)
            nc.sync.dma_start(
                z_sb[:, 0:8, :],
                z_d[0:1024, :].rearrange("(t p) l -> p t l", p=128),
            )
            w_row = cpool.tile([1, K], F32)
            nc.sync.dma_start(w_row, w_d[:].rearrange("(a k) -> a k", a=1))
            W1_sb = cpool.tile([L, H], F32)
            nc.sync.dma_start(W1_sb, W1_d[:, :])
            b2_row = cpool.tile([1, D], F32)
            nc.sync.dma_start(b2_row, b2_d[:].rearrange("(a d) -> a d", a=1))
            b1_col = cpool.tile([H, 1], F32)
            nc.sync.dma_start(b1_col, b1_d[:].rearrange("(h a) -> h a", a=1))
            x_sb = cpool.tile([128, BT, D], F32)
            nc.sync.dma_start(
                x_sb[:, 0:2, :],
                x_d[0:256, :].rearrange("(t p) d -> p t d", p=128),
            )
            nc.sync.dma_start(
                z_sb[:, 8:16, :],
                z_d[1024:2048, :].rearrange("(t p) l -> p t l", p=128),
            )
            for q in range(1, 4):
                nc.sync.dma_start(
                    x_sb[:, 2 * q : 2 * (q + 1), :],
                    x_d[256 * q : 256 * (q + 1), :].rearrange("(t p) d -> p t d", p=128),
                )
            # SWDGE (Pool) path just for W2 (feeds the M chain in parallel)
            W2_sb = cpool.tile([H, D], F32)
            nc.gpsimd.dma_start(W2_sb, W2_d[:, :])

            # ---------------- persistent tensors ----------------
            zT = cpool.tile([128, KT, L], BF16)  # [l, kt, k]
            W2aug = cpool.tile([HA, DT, 128], BF16)
            W2augT = cpool.tile([128, DT, HA], BF16)
            M_bf = cpool.tile([HA, HA], BF16)
            W1_bf = cpool.tile([L, H], BF16)
            qfaug = cpool.tile([HA, K], BF16)  # rows 0..64 = h~ .* Mh~
            lnw_row = cpool.tile([1, K], BF16)  # ln w
            hTaug = cpool.tile([HE, K], BF16)  # relu; ones; 0.5(ln w - p2)
            yTaug = cpool.tile([HE, BS], BF16)  # [j, b]; row 65 ones
            negx2 = cpool.tile([128, BT], F32)
            racc = cpool.tile([128, 2 * BT], F32)

            # ---------------- emission in expected execution order ---------
            # Per-engine instruction streams run strictly in emission order,
            # so ops are emitted in the order they are expected to become
            # ready: z chunk chains as z halves land, W2aug/M prep between,
            # x tiles as x quarters land, then main-loop passes interleaved
            # with the remaining x tiles.
            relu_eng = {1: nc.vector, 2: nc.vector, 3: nc.vector}
            zt_eng = {0: nc.scalar, 1: nc.vector, 2: nc.vector, 3: nc.vector}
            qf_eng = {0: nc.vector, 1: nc.vector, 2: nc.vector, 3: nc.vector}

            def z_transpose(c):
                ptz = psp.tile([128, 4, 128], F32, tag="sm", bufs=4, name=f"ptz{c}")
                for j in range(4):
                    nc.tensor.transpose(ptz[:, j, :], z_sb[:, 4 * c + j, :], ident)
                if c == 0:
                    nc.scalar.copy(zT[:, 4 * c : 4 * (c + 1), :], ptz)
                else:
                    zt_eng[c].tensor_copy(zT[:, 4 * c : 4 * (c + 1), :], ptz)

            def h_chunk(c):
                ks = slice(512 * c, 512 * (c + 1))
                ph = psp.tile([H, 512], F32, tag="sm", bufs=4, name=f"ph{c}")
                nc.tensor.matmul(
                    ph, W1_bf, zT[:, 4 * c : 4 * (c + 1), :], start=True, stop=True
                )
                if c <= 1:
                    nc.scalar.activation(
                        hTaug[0:H, ks], ph, AF.Relu, bias=b1_col, scale=1.0
                    )
                else:
                    relu_eng[c].tensor_scalar(
                        hTaug[0:H, ks], ph, b1_col, 0.0,
                        op0=mybir.AluOpType.add, op1=mybir.AluOpType.max,
                    )

            def bias_chunk(c):
                ks = slice(512 * c, 512 * (c + 1))
                pmh = psp.tile([HA, 512], F32, tag="sm", bufs=4, name=f"pmh{c}")
                nc.tensor.matmul(pmh, M_bf, hTaug[0:HA, ks], start=True, stop=True)
                qf_eng[c].scalar_tensor_tensor(
                    qfaug[:, ks], pmh, 1.0, hTaug[0:HA, ks],
                    op0=mybir.AluOpType.mult, op1=mybir.AluOpType.mult,
                )
                pb2 = psp.tile([2, 512], F32, tag="sm", bufs=4, name=f"pb2_{c}")
                nc.tensor.matmul(pb2, onesel, ones_row, start=True, stop=False)
                nc.tensor.matmul(pb2, neghalf2, qfaug[:, ks], start=False, stop=False)
                nc.tensor.matmul(pb2, possel, lnw_row[:, ks], start=False, stop=True)
                if c < 2:
                    nc.scalar.copy(hTaug[H:HE, ks], pb2)
                elif c == 2:
                    nc.vector.tensor_copy(hTaug[H:HE, ks], pb2)
                else:
                    nc.vector.tensor_copy(hTaug[H:HE, ks], pb2)

            def x_tile(t):
                xsq = wpool.tile([128, D], BF16, tag="xsq", name=f"xsq{t}")
                x16 = wpool.tile([128, D], BF16, tag="x16", name=f"x16_{t}")
                nc.vector.tensor_copy(x16, x_sb[:, t, :])
                ptx = psp.tile([128, DT, 128], BF16, tag="sm", bufs=4, name=f"ptx{t}")
                for d in range(DT):
                    nc.tensor.transpose(
                        ptx[:, d, :], x16[:, 128 * d : 128 * (d + 1)], ident_bf
                    )
                xTt = wpool.tile([128, DT, 128], BF16, tag="xT", name=f"xT{t}")
                nc.vector.tensor_copy(xTt, ptx)
                pyT = psp.tile([HA, 128], F32, tag="sm", bufs=4, name=f"pyT{t}")
                for d in range(DT):
                    nc.tensor.matmul(
                        pyT, W2augT[:, d, :], xTt[:, d, :],
                        start=(d == 0), stop=(d == DT - 1),
                    )
                nc.vector.tensor_copy(yTaug[0:HA, 128 * t : 128 * (t + 1)], pyT)
                # -x2 fused: (x * -1) * x with accumulate
                nc.vector.scalar_tensor_tensor(
                    xsq, x_sb[:, t, :], -1.0, x_sb[:, t, :],
                    op0=mybir.AluOpType.mult, op1=mybir.AluOpType.mult,
                    accum_out=negx2[:, t : t + 1],
                )

            def x_pair(t):
                # two b-tiles share one PSUM bank for transposes and one for
                # yT: halves the "sm" slot pressure for the late tiles
                x16 = wpool.tile([128, 2, D], BF16, tag="x16p", name=f"x16p{t}")
                nc.gpsimd.tensor_copy(x16[:, 0, :], x_sb[:, t, :])
                nc.gpsimd.tensor_copy(x16[:, 1, :], x_sb[:, t + 1, :])
                ptx = psp.tile([128, 2, DT, 128], BF16, tag="sm", bufs=4, name=f"ptxp{t}")
                for j in range(2):
                    for d in range(DT):
                        nc.tensor.transpose(
                            ptx[:, j, d, :], x16[:, j, 128 * d : 128 * (d + 1)], ident_bf
                        )
                xTt = wpool.tile([128, 2, DT, 128], BF16, tag="xTp", name=f"xTp{t}")
                nc.vector.tensor_copy(xTt, ptx)
                pyT = psp.tile([HA, 2, 128], F32, tag="sm", bufs=4, name=f"pyTp{t}")
                for j in range(2):
                    for d in range(DT):
                        nc.tensor.matmul(
                            pyT[:, j, :], W2augT[:, d, :], xTt[:, j, d, :],
                            start=(d == 0), stop=(d == DT - 1),
                        )
                nc.vector.tensor_copy(yTaug[0:HA, 128 * t : 128 * (t + 2)], pyT)
                for j in range(2):
                    xsq = wpool.tile([128, D], BF16, tag="xsq", name=f"xsq{t+j}")
                    nc.vector.scalar_tensor_tensor(
                        xsq, x_sb[:, t + j, :], -1.0, x_sb[:, t + j, :],
                        op0=mybir.AluOpType.mult, op1=mybir.AluOpType.mult,
                        accum_out=negx2[:, t + j : t + j + 1],
                    )

            def main_pass(hlf, t):
                bsl = slice(128 * t, 128 * (t + 1))
                pg = psp.tile([128, 1024], F32, tag="big", bufs=2, name=f"pg{t}_{hlf}")
                for c2 in range(2):
                    c = 2 * hlf + c2
                    ks = slice(512 * c, 512 * (c + 1))
                    nc.tensor.matmul(
                        pg[:, 512 * c2 : 512 * (c2 + 1)],
                        yTaug[:, bsl],
                        hTaug[:, ks],
                        start=True,
                        stop=True,
                    )
                U = wpool.tile([128, 1024], BF16, tag="U", name=f"U{t}_{hlf}")
                nc.scalar.activation(
                    U,
                    pg,
                    AF.Exp,
                    bias=negx2[:, t : t + 1],
                    scale=2.0,
                    accum_out=racc[:, 2 * t + hlf : 2 * t + hlf + 1],
                )

            # ones rows: no upstream deps. yTaug rows 64:66 start as ones;
            # the per-tile yT copies later overwrite row 64 with x.b2
            nc.vector.memset(hTaug[H : H + 1, :], 1.0)
            nc.gpsimd.memset(yTaug[H:HE, :], 1.0)

            # t~2.1: z half 0 -> chunk 0 transposes; W1 cast
            z_transpose(0)
            nc.vector.tensor_copy(W1_bf, W1_sb)
            # W-block: W2aug -> W2augT -> M (W2 ~2.5 SWDGE, b2 ~3.4 HWDGE)
            nc.vector.tensor_copy(
                W2aug[0:H, :, :], W2_sb.rearrange("h (t d) -> h t d", d=128)
            )
            nc.vector.tensor_copy(
                W2aug[H:HA, :, :], b2_row.rearrange("a (t d) -> a t d", d=128)
            )
            # 66-wide so each bf16 d-slice stays 4-byte aligned in PSUM
            h_chunk(0)
            ptw = psp.tile([128, DT, HA + 1], BF16, tag="sm", bufs=4, name="ptw")
            for d in range(DT):
                nc.tensor.transpose(ptw[:, d, 0:HA], W2aug[:, d, :], ident_bf[:HA, :HA])
            nc.vector.tensor_copy(W2augT, ptw[:, :, 0:HA])
            z_transpose(1)
            h_chunk(1)
            pm = psp.tile([HA, HA], F32, tag="sm", bufs=4, name="pm")
            for d in range(DT):
                nc.tensor.matmul(
                    pm, W2augT[:, d, :], W2augT[:, d, :], start=(d == 0), stop=(d == DT - 1)
                )
            nc.vector.tensor_copy(M_bf, pm)
            nc.scalar.activation(
                lnw_row[:, 0:1024], w_row[:, 0:1024], AF.Ln
            )
            bias_chunk(0)
            x_tile(0)
            bias_chunk(1)
            x_tile(1)
            nc.scalar.activation(
                lnw_row[:, 1024:2048], w_row[:, 1024:2048], AF.Ln
            )
            main_pass(0, 0)
            # t~8.3: z half 1 -> chunks 2/3 (bias copies off ACT: it exps now)
            x_pair(2)
            main_pass(0, 1)
            z_transpose(2)
            h_chunk(2)
            x_pair(4)
            main_pass(0, 2)
            z_transpose(3)
            h_chunk(3)
            x_pair(6)
            main_pass(0, 3)
            bias_chunk(2)
            main_pass(0, 4)
            bias_chunk(3)
            main_pass(0, 5)
            main_pass(0, 6)
            main_pass(0, 7)
            for t in range(BT):
                main_pass(1, t)

            # ---------------- ship per-row sums; host sums / B ----------------
            # split so most of the DMA overlaps the last exp passes
            nc.sync.dma_start(out_d[:, 0:8], racc[:, 0:8])
            nc.sync.dma_start(out_d[:, 8:16], racc[:, 8:16])

    nc.finalize()
    return nc


_NC_CACHE = None


def _get_nc() -> bass.Bass:
    global _NC_CACHE
    if _NC_CACHE is None:
        _NC_CACHE = build_nc()
    return _NC_CACHE


def kernel(x, z_samples, w, W1, b1, W2, b2, _trace=False):
    x = np.ascontiguousarray(np.asarray(x, dtype=np.float32))
    z_samples = np.ascontiguousarray(np.asarray(z_samples, dtype=np.float32))
    w = np.ascontiguousarray(np.asarray(w, dtype=np.float32))
    W1 = np.ascontiguousarray(np.asarray(W1, dtype=np.float32))
    b1 = np.ascontiguousarray(np.asarray(b1, dtype=np.float32))
    W2 = np.ascontiguousarray(np.asarray(W2, dtype=np.float32))
    b2 = np.ascontiguousarray(np.asarray(b2, dtype=np.float32))

    nc = _get_nc()
    in_maps = [
        {
            "x": x[i * BS : (i + 1) * BS],
            "z_samples": z_samples,
            "w": w,
            "W1": W1,
            "b1": b1,
            "W2": W2,
            "b2": b2,
        }
        for i in range(N_CORES)
    ]
    res = run_bass_kernel_spmd(nc, in_maps, core_ids=list(range(N_CORES)), trace=_trace)
    total = sum(float(np.sum(np.asarray(r["out"], dtype=np.float64))) for r in res.results)
    out = np.array(total / B, dtype=np.float32)
    if _trace:
        return out, res
    return out
